# revision 21
# baseline (speedup 1.0000x reference)
"""Causal self-attention (B=2, T=2048, C=1024, H=16) on 8 TRN2 NeuronCores.

Sharding: core c -> batch b = c // 4, head-group hg = c % 4 (4 heads each).
Each core computes q,k,v for its 4 heads, causal attention, and a partial
output projection (its 256 rows of w_proj). Host sums the 4 partials per
batch.

On-chip layout is fully "transposed" so no on-chip transposes are needed:
  - host passes xT = x[b].T  [C, T]
  - qT, kT computed as [head*64, T] (head-dim on partitions)
  - v computed as [T, head*65] where the 65th column per head is ones
  - scores computed transposed: sT[keys, queries] = kT_h^T-chunk @ qT_h
  - exp on ScalarE (no max subtraction: |logits/8| <= ~8, exp is safe in f32)
  - causal: fully-masked key-chunks skipped; diagonal chunks multiplied by a
    precomputed 0/1 band mask
  - PV matmul lhsT = v_aug[jchunk, head] [128, 65]: rows 0..63 accumulate
    y^T, row 64 (ones) accumulates the softmax denominator -- one matmul
  - normalize: reciprocal of the denominator row, broadcast across 64
    partitions with a stride-0 DMA, multiply on VectorE
  - projection consumes y^T [c_in, t] directly as lhsT

Exact bias simplifications: the k-bias is dropped entirely (a constant
shift along the key axis cancels in softmax), and the v-bias is applied on
the host as y += bv @ w_proj (softmax rows sum to 1), which removes the
per-t-chunk bias matmul from the v chains (-16 matmuls/core).

All matmul operands are bfloat16 (DRAM inputs are converted on the host in
make_in_maps): same PE stream rate as float32r but half the DMA/SBUF
traffic, 2-byte weight loads, and 2x DVE throughput. Measured rel err
3.9e-3 (gate 2e-2). Measured on HW: the kernel's bare 640-matmul stream
takes ~73us; the ScalarE exp stream (~75us incl. per-inst overheads) is the
co-bottleneck, so elementwise work placement, not matmul count, bounds
further gains.
"""

import numpy as np

import concourse.bacc as bacc
import concourse.mybir as mybir
import concourse.tile as tile
from concourse.bass_utils import run_bass_kernel_spmd

P = 128           # partitions
T = 2048          # sequence length
C = 1024          # model dim
NHC = 4           # heads per core
HD = 64           # head dim
JW = NHC * HD     # 256 qkv columns per core
VW = NHC * (HD + 1)  # 260: v + ones column per head
NCC = C // P      # 8 contraction chunks over C
NT = T // P       # 16 key/t chunks of 128
FI = 512          # query chunk (free dim of score matmuls)
NI = T // FI      # 4 query chunks

F32 = mybir.dt.float32
EXPF = mybir.ActivationFunctionType.Exp
COPYF = mybir.ActivationFunctionType.Copy
IDENT = mybir.ActivationFunctionType.Identity

# Delayed-projection placement: which windows' projections are used as PE
# fill work inside which later window's attention loops.
PROJ_SCHED = {2: [0], 3: [1, 2]}

# Matmul input dtype: bfloat16 streams 1 row/cycle on the PE (same rate as
# float32r) but with 2-byte storage: half the DMA/SBUF traffic, a 2-byte
# LDWEIGHTS path (fp32r's 4-byte weight load is ~2x slower and hard to hide
# behind a 512-col stream), and 2x DVE throughput on elementwise ops.
# Accuracy measured on CPU: all-bf16 rel err 3.5e-3 (gate 2e-2).
MM_DT = mybir.dt.bfloat16

# Storage numpy dtype for a given matmul dtype: float32r is a bitcast view of
# f32 bytes; bf16 is a real 2-byte format converted on the host.
def _np_dt(mm_dt):
    if mm_dt in (mybir.dt.float32, mybir.dt.float32r):
        return np.float32
    return mybir.dt.np(mm_dt)


def build_nc(mm_dt=MM_DT, interleave="fine", proj_pool=False,
             mm_bufs=2, s_bufs=2, p_bufs=4, o_bufs=4, reps=1,
             pace=0.75, bcs_q="sp", y_q="sp", xt_g=4,
             split_ss=False, pv_bufs=2, xt0_fine=False, fast_start=True,
             fuse_norm=False, proj_sched=None, qk_dve=False,
             win_order=None, xt0_q="sp", attn_la=2, weights_q="sp",
             win0_split=False):
    nc = bacc.Bacc(
        "TRN2", target_bir_lowering=False, debug=False, enable_asserts=True
    )

    # DRAM storage dtype for matmul operands: real 2-byte tensors for bf16
    # (host converts in make_in_maps), f32 bytes bitcast for float32/f32r.
    SD = F32 if mm_dt in (F32, mybir.dt.float32r) else mm_dt

    xt_d = nc.dram_tensor("xt", [C, T], SD, kind="ExternalInput")
    wq_d = nc.dram_tensor("wq", [C, JW], SD, kind="ExternalInput")
    wk_d = nc.dram_tensor("wk", [C, JW], SD, kind="ExternalInput")
    wv_d = nc.dram_tensor("wv", [C, JW], SD, kind="ExternalInput")
    bq_d = nc.dram_tensor("bq", [JW], F32, kind="ExternalInput")
    wp_d = nc.dram_tensor("wp", [JW, C], SD, kind="ExternalInput")
    mask_d = nc.dram_tensor("mask_c", [P, 2 * FI], SD, kind="ExternalInput")
    vones_d = nc.dram_tensor("vones_c", [P, NHC], SD, kind="ExternalInput")
    y_d = nc.dram_tensor("y", [T, C], F32, kind="ExternalOutput")

    # Tiles that feed matmuls are declared in the matmul dtype (the BIR
    # verifier requires every producer of an FP32r matmul operand to emit
    # float32r; for bf16 the tiles genuinely store 2-byte values).
    MMD = mm_dt

    def r(ap):  # matmul-operand view of an AP: ensure dtype == mm_dt
        if mm_dt == F32 or ap.dtype == mm_dt:
            return ap
        return ap.bitcast(mm_dt)

    with tile.TileContext(nc) as tc, \
            nc.allow_low_precision(reason="fp32r matmul operand tiles"):
        with (
            tc.tile_pool(name="big", bufs=1) as big,
            tc.tile_pool(name="pp", bufs=p_bufs) as p_pool,
            tc.tile_pool(name="op", bufs=o_bufs) as o_pool,
            tc.tile_pool(name="rows", bufs=2) as row_pool,
            tc.tile_pool(name="psmm", bufs=mm_bufs, space="PSUM") as ps_mm,
            # pair score tiles are [P, 2*FI] = 2 banks each
            tc.tile_pool(name="pss",
                         bufs=s_bufs if s_bufs is not None else 1,
                         space="PSUM") as ps_s,
            tc.tile_pool(name="pspv", bufs=pv_bufs, space="PSUM") as ps_pv,
            tc.tile_pool(name="psbc", bufs=1, space="PSUM") as ps_bc,
            tc.tile_pool(name="pspj", bufs=1, space="PSUM") as ps_pj_real,
        ):
            ps_pj = ps_pj_real if proj_pool else ps_mm
            ps_bcp = ps_bc if proj_pool else ps_mm
            bc_tag = "bc" if proj_pool else "mm"
            # --- constant/weight loads, emitted lazily in compute-gated
            # order (first-window xT and wq first; wp only before proj) ---
            consts = {}
            dma_q = {"sp": nc.sync.dma_start, "gp": nc.gpsimd.dma_start,
                     "dve": nc.vector.dma_start, "act": nc.scalar.dma_start}

            def load_w(nm, d, store, ng=2):
                # ng tiles, NCC/ng contraction chunks each
                cpg = NCC // ng
                store["cpg"] = cpg
                for g in range(ng):
                    load_w_chunk(nm, d, store, g, cpg)

            def load_w_chunk(nm, d, store, g, cpg=1):
                store.setdefault("cpg", cpg)
                wt = big.tile([P, cpg * JW], MMD, tag=f"{nm}{g}",
                              name=f"{nm}{g}")
                ap = d.ap()[cpg * g * P:(cpg * g + cpg) * P, :]
                dma = dma_q[weights_q]
                if cpg == 1:
                    dma(wt[:], r(ap))
                else:
                    dma(
                        wt.rearrange("p (c j) -> p c j", c=cpg),
                        r(ap.rearrange("(c p) j -> p c j", p=P)),
                    )
                store[g] = wt

            def w_sl(store, ci, lo, hi):
                # [P, hi-lo] slice of contraction chunk ci's columns
                cpg = store["cpg"]
                g, c = ci // cpg, ci % cpg
                return store[g][:, c * JW + lo:c * JW + hi]

            def load_biases():
                # bk is dropped exactly (a per-(t,head) constant shift along
                # keys leaves softmax unchanged); bv is folded into the host
                # combine (softmax rows sum to 1 -> y += bv @ w_proj).
                for kc in range(2):
                    bqt = big.tile([P, 1], F32, tag=f"bq{kc}", name=f"bq{kc}")
                    dma_q[weights_q](
                        bqt[:],
                        bq_d.ap()[kc * P:(kc + 1) * P]
                        .rearrange("(p o) -> p o", o=1),
                    )
                    bq_t[kc] = bqt

            def load_consts():
                # maskb[j, u] = 1 if u >= j else 0, duplicated for head pairs
                maskb = big.tile([P, 2 * FI], MMD, tag="maskb", name="maskb")
                dma_q[weights_q](maskb[:], r(mask_d.ap()[:, :]))
                consts["maskb2"] = maskb.rearrange("p (h f) -> p h f", h=2)
                vones = big.tile([P, NHC], MMD, tag="vones", name="vones")
                dma_q[weights_q](vones[:], r(vones_d.ap()[:, :]))
                consts["vones"] = vones

            def load_wp():
                for kc in range(2):
                    wpt = big.tile([P, C], MMD, tag=f"wp{kc}", name=f"wp{kc}")
                    dma_q[weights_q](wpt[:],
                                     r(wp_d.ap()[kc * P:(kc + 1) * P, :]))
                    wp_t[kc] = wpt

            bq_t = {}
            wq_t, wk_t, wv_t, wp_t = {}, {}, {}, {}

            yT = {}
            for kc in range(2):
                for ic in range(NI):
                    yt = big.tile([P, FI], MMD, tag=f"yT{kc}_{ic}",
                                  name=f"yT{kc}_{ic}")
                    yT[(kc, ic)] = yt

            xt_t, qT, kT, v_t = {}, {}, {}, {}

            def xt_groups(ic):
                return 8 if ((xt0_fine or fast_start) and ic == 0) else xt_g

            def emit_xt_chunk(ic, g, cpg):
                xtt = big.tile([P, cpg * FI], MMD, tag=f"xt{g}_{ic}",
                               name=f"xt{g}_{ic}")
                ap = xt_d.ap()[cpg * g * P:(cpg * g + cpg) * P,
                               ic * FI:(ic + 1) * FI]
                dma = dma_q[xt0_q] if ic == 0 else nc.sync.dma_start
                if cpg == 1:
                    dma(xtt[:], r(ap))
                else:
                    dma(
                        xtt.rearrange("p (c u) -> p c u", c=cpg),
                        r(ap.rearrange("(c p) u -> p c u", p=P)),
                    )
                xt_t[(g, ic)] = xtt

            def emit_xt_dma(ic):
                # xT for this t-window: xt_g DMAs, cpg contraction chunks each
                ng = xt_groups(ic)
                cpg = NCC // ng
                for g in range(ng):
                    emit_xt_chunk(ic, g, cpg)

            def xt_sl(ci, ic, lo, hi):
                cpg = NCC // xt_groups(ic)
                g, c = ci // cpg, ci % cpg
                return xt_t[(g, ic)][:, c * FI + lo:c * FI + hi]

            def emit_qk_one(ic, which, kc):
                nm, w_t, store = (("qT", wq_t, qT), ("kT", wk_t, kT))[which]
                ps = ps_mm.tile([P, FI], F32, tag="mm", name="ps_qk")
                for ci in range(NCC):
                    nc.tensor.matmul(
                        ps[:],
                        r(w_sl(w_t, ci, kc * P, (kc + 1) * P)),
                        r(xt_sl(ci, ic, 0, FI)),
                        start=(ci == 0),
                        stop=(ci == NCC - 1),
                    )
                st = big.tile([P, FI], MMD, tag=f"{nm}{kc}_{ic}",
                              name=f"{nm}{kc}_{ic}")
                if qk_dve:
                    # keep ScalarE free for the exp stream: bias-add/copy on
                    # DVE ([P,1] per-partition scalar add)
                    if which == 0:
                        nc.vector.tensor_scalar_add(st[:], ps[:],
                                                    bq_t[kc][:])
                    else:
                        nc.vector.tensor_copy(st[:], ps[:])
                elif which == 0:
                    nc.scalar.activation(st[:], ps[:], IDENT,
                                         bias=bq_t[kc][:], scale=1.0)
                else:
                    nc.scalar.activation(st[:], ps[:], IDENT, scale=1.0)
                store[(kc, ic)] = st

            def emit_qk(ic, which):
                for kc in range(2):
                    emit_qk_one(ic, which, kc)

            def emit_v(ic, half):
                for tc_i in range(4 * ic + 2 * half, 4 * ic + 2 * half + 2):
                    emit_v_one(ic, tc_i)

            def emit_v_one(ic, tc_i):
                if True:
                    ps = ps_mm.tile([P, JW], F32, tag="mm", name="ps_v")
                    for ci in range(NCC):
                        nc.tensor.matmul(
                            ps[:],
                            r(xt_sl(ci, ic, (tc_i % 4) * P,
                                    (tc_i % 4 + 1) * P)),
                            r(w_sl(wv_t, ci, 0, JW)),
                            start=(ci == 0),
                            stop=(ci == NCC - 1),
                        )
                    vt = big.tile([P, VW], MMD, tag=f"v{tc_i}",
                                  name=f"v{tc_i}")
                    vt3 = vt.rearrange("p (h e) -> p h e", e=HD + 1)
                    nc.vector.tensor_copy(
                        vt3[:, :, 0:HD],
                        ps.rearrange("p (h e) -> p h e", e=HD),
                    )
                    nc.vector.tensor_copy(
                        vt3[:, :, HD:HD + 1],
                        consts["vones"].rearrange("p (h o) -> p h o", o=1),
                    )
                    v_t[tc_i] = vt

            def gen_fill_units(qkv_list, proj_list):
                # small PE work units interleaved into attention chunk loops
                for icn in qkv_list:
                    for which in range(2):
                        for kc in range(2):
                            emit_qk_one(icn, which, kc)
                            yield True
                    for tc_i in range(4 * icn, 4 * (icn + 1)):
                        emit_v_one(icn, tc_i)
                        yield True
                for ic_proj in proj_list:
                    for tc_i in range(4 * ic_proj, 4 * (ic_proj + 1)):
                        emit_proj_one(ic_proj, tc_i)
                        yield True

            def emit_qkv_piece(ic, piece):
                if piece == 0:
                    emit_qk(ic, 0)
                elif piece == 1:
                    emit_qk(ic, 1)
                else:
                    emit_v(ic, piece - 2)

            def emit_attention_pair(ic, hp, fill=None, per_cp=0.0,
                                    la=attn_la):
                # attention for query window ic, heads (2*hp, 2*hp+1): both
                # live in partition rows of the kc=hp qT/kT tiles, so their
                # score chunks share one [P, 2*FI] psum tile and ONE exp and
                # mask op each ([P, 2, w] strided APs).
                #
                # Software-pipelined: scores run `la` chunks ahead of the PV
                # matmuls in PE program order, so the in-order PE has score
                # work queued while PV(jc) waits out the exp+mask chain
                # (~1.4us) instead of stalling every chunk.
                kc = hp
                njc = 4 * (ic + 1)
                pv = {}
                for sub in range(2):
                    pv[sub] = ps_pv.tile([HD + 1, FI], F32, tag="pv",
                                         name="ps_pv")

                def emit_score(jc):
                    rr = jc * P - ic * FI  # key offset into query window
                    w = FI - rr if rr > 0 else FI  # valid column suffix
                    pt = p_pool.tile([P, 2 * FI], MMD, tag="p", name="p_t")
                    pt3 = pt.rearrange("p (h f) -> p h f", h=2)
                    ss = ps_s.tile([P, 2 * FI], F32, tag="s", name="ps_s")
                    for sub in range(2):
                        nc.tensor.matmul(
                            ss[:, sub * FI:sub * FI + w],
                            r(kT[(kc, jc // 4)][sub * HD:(sub + 1) * HD,
                                                (jc % 4) * P:
                                                (jc % 4 + 1) * P]),
                            r(qT[(kc, ic)][sub * HD:(sub + 1) * HD,
                                           FI - w:]),
                            start=True,
                            stop=True,
                        )
                    ss3 = ss.rearrange("p (h f) -> p h f", h=2)
                    nc.scalar.activation(pt3[:, :, :w], ss3[:, :, :w],
                                         EXPF, scale=0.125)
                    if rr >= 0:  # diagonal chunk: zero future keys
                        nc.vector.tensor_mul(
                            pt3[:, :, :w], pt3[:, :, :w],
                            consts["maskb2"][:, :, :w]
                        )
                    return pt, w

                def emit_pv(ji, jc, pt, w):
                    for sub in range(2):
                        hh = 2 * hp + sub
                        nc.tensor.matmul(
                            pv[sub][:, FI - w:],
                            r(v_t[jc][:, hh * (HD + 1):(hh + 1) * (HD + 1)]),
                            r(pt[:, sub * FI:sub * FI + w]),
                            start=(ji == 0),
                            stop=(ji == njc - 1),
                            skip_group_check=True,
                        )

                pts = {}
                for jc in range(min(la, njc)):
                    pts[jc] = emit_score(jc)
                credit = 0.0
                for ji, jc in enumerate(range(njc)):
                    if fill is not None:
                        credit += per_cp
                        while credit >= 1.0:
                            credit -= 1.0
                            if next(fill, None) is None:
                                credit = 0.0
                                break
                    if jc + la < njc:
                        pts[jc + la] = emit_score(jc + la)
                    elif la == 0:
                        pts[jc] = emit_score(jc)
                    pt, w = pts.pop(jc)
                    emit_pv(ji, jc, pt, w)
                # broadcast each head's reciprocal row across its 64
                # head-dim partitions with a stride-0-source DMA: keeps the
                # PE and the shared mm psum pool out of the normalize chain.
                # bcs spans all 128 partitions so the SB+SB tensor_mul sees
                # equal base partitions (walrus checkSBSameStartPartition).
                bcs = row_pool.tile([P, FI], F32, tag="bcs", name="bcs")
                for sub in range(2):
                    po = sub * HD
                    rrow = row_pool.tile([1, FI], F32, tag="rr", name="rrow")
                    nc.vector.reciprocal(rrow[:], pv[sub][HD:HD + 1, :])
                    dma_q[bcs_q](
                        bcs[po:po + HD, :],
                        rrow[0:1, :].rearrange("(o b) f -> o b f", b=1)
                        .broadcast_to([1, HD, FI]),
                    )
                    ysl = yT[(kc, ic)][po:po + HD, :]
                    if fuse_norm:
                        # single DVE pass: yT = pv * (1/den) straight from
                        # PSUM (drops the intermediate copy)
                        nc.vector.tensor_mul(ysl, pv[sub][0:HD, :],
                                             bcs[po:po + HD, :])
                    else:
                        nc.vector.tensor_copy(ysl, pv[sub][0:HD, :])
                        nc.vector.tensor_mul(ysl, ysl, bcs[po:po + HD, :])

            def emit_proj(ic):
                # projection for this query window (t chunks 4*ic .. 4*ic+3)
                for tc_i in range(4 * ic, 4 * (ic + 1)):
                    emit_proj_one(ic, tc_i)

            def emit_proj_one(ic, tc_i):
                # one [P, C] output tile and one DMA per t-chunk
                if True:
                    tof = (tc_i % 4) * P
                    ot = o_pool.tile([P, C], F32, tag="o", name="o_t")
                    for n2 in range(2):
                        ps = ps_pj.tile([P, FI], F32,
                                        tag="pj" if proj_pool else "mm",
                                        name="ps_o")
                        for kc in range(2):
                            nc.tensor.matmul(
                                ps[:],
                                r(yT[(kc, ic)][:, tof:tof + P]),
                                r(wp_t[kc][:, n2 * FI:(n2 + 1) * FI]),
                                start=(kc == 0),
                                stop=(kc == 1),
                            )
                        nc.vector.tensor_copy(
                            ot[:, n2 * FI:(n2 + 1) * FI], ps[:])
                        if ic == NI - 1:
                            # last window: split the drain so the final DMA
                            # is half-size and starts after the first copy
                            dma_q[y_q](
                                y_d.ap()[tc_i * P:(tc_i + 1) * P,
                                         n2 * FI:(n2 + 1) * FI],
                                ot[:, n2 * FI:(n2 + 1) * FI])
                    if ic != NI - 1:
                        dma_q[y_q](
                            y_d.ap()[tc_i * P:(tc_i + 1) * P, :], ot[:])

            def emit_qkv(ic):
                emit_xt_dma(ic)
                for piece in range(4):
                    emit_qkv_piece(ic, piece)

            def load_front():
                load_biases()
                load_w("wq", wq_d, wq_t)
                load_w("wk", wk_d, wk_t)
                load_w("wv", wv_d, wv_t)
                load_consts()

            for _rep in range(reps):
                if interleave == "fine":
                    # QKV(ic+1) pieces slotted between attention pairs of
                    # window ic: PE fill work while ScalarE runs exp. proj is
                    # delayed one window so the last window (which has no
                    # QKV left) still gets PE fill between its pairs.
                    if _rep == 0:
                        if fast_start:
                            # interleave first-window xt chunks with wq
                            # chunks: the first q chain starts after ~0.5MB
                            # of DMA instead of ~3MB. Biases are tiny and
                            # only needed at the first copy-out, so they go
                            # after the first chunk pair.
                            for ciq in range(NCC):
                                emit_xt_chunk(0, ciq, 1)
                                load_w_chunk("wq", wq_d, wq_t, ciq)
                                if ciq == 0:
                                    load_biases()
                            load_w("wk", wk_d, wk_t)
                            load_w("wv", wv_d, wv_t)
                            load_consts()
                        else:
                            emit_xt_dma(0)
                            load_front()
                        if win0_split:
                            # only what attention(0) hp=0 needs before the
                            # first pair: q/k for kc=0 and all 4 v chunks;
                            # the kc=1 chains become fill inside hp=0
                            emit_qk_one(0, 0, 0)
                            emit_qk_one(0, 1, 0)
                            emit_v(0, 0)
                            emit_v(0, 1)
                        else:
                            for piece in range(4):
                                emit_qkv_piece(0, piece)
                        load_wp()
                    else:
                        emit_qkv(0)
                    # Window schedule: attention order, per-window QKV fill
                    # lists, delayed-projection fill lists, and the tail
                    # projection. "0132" runs the full window 3 before the
                    # shorter window 2, shrinking the un-overlapped tail.
                    if win_order == "0132":
                        sched = [(0, [1], []), (1, [2, 3], []),
                                 (3, [], [0, 1]), (2, [], [3])]
                        tail = [2]
                    else:
                        if proj_sched is None:
                            proj_sched = PROJ_SCHED if NI == 4 else {
                                ic: [ic - 1] for ic in range(1, NI)}
                        sched = [
                            (ic, [ic + 1] if ic + 1 < NI else [],
                             proj_sched.get(ic, []))
                            for ic in range(NI)
                        ]
                        tail = [NI - 1]
                    for wi, (ic, qkv_list, proj_list) in enumerate(sched):
                        for icn in qkv_list:
                            emit_xt_dma(icn)
                        n_units = 8 * len(qkv_list) + 4 * len(proj_list)
                        fill = gen_fill_units(qkv_list, proj_list)
                        prefill = None
                        if win0_split and _rep == 0 and wi == 0:
                            # window-0 kc=1 q/k chains ride as fill in hp=0;
                            # force-drained before hp=1 (which reads them)
                            def _kc1_units():
                                emit_qk_one(0, 0, 1)
                                yield True
                                emit_qk_one(0, 1, 1)
                                yield True
                            prefill = _kc1_units()
                            import itertools
                            fill = itertools.chain(prefill, fill)
                            n_units += 2
                        pc = pace[ic] if isinstance(pace, dict) else pace
                        per_cp = pc * n_units / (2.0 * 4 * (ic + 1))
                        for hp in range(2):
                            emit_attention_pair(ic, hp, fill, per_cp)
                            if hp == 0 and prefill is not None:
                                for _ in prefill:  # ensure kc=1 ready
                                    pass
                        for _ in fill:  # drain leftovers
                            pass
                    for ic_t in tail:
                        emit_proj(ic_t)
                elif interleave:
                    if _rep == 0:
                        emit_xt_dma(0)
                        load_front()
                        load_wp()
                    for ic in range(NI):
                        if _rep == 0 and ic == 0:
                            for piece in range(4):
                                emit_qkv_piece(0, piece)
                        else:
                            emit_qkv(ic)
                        for hp in range(2):
                            emit_attention_pair(ic, hp)
                        emit_proj(ic)
                else:
                    if _rep == 0:
                        emit_xt_dma(0)
                        load_front()
                        load_wp()
                    for ic in range(NI):
                        if _rep == 0 and ic == 0:
                            for piece in range(4):
                                emit_qkv_piece(0, piece)
                        else:
                            emit_qkv(ic)
                    for ic in range(NI):
                        for hp in range(2):
                            emit_attention_pair(ic, hp)
                        emit_proj(ic)



    nc.compile()
    return nc


_NC_CACHE = {}


def _get_nc(mm_dt=MM_DT, **kw):
    key = (str(mm_dt), tuple(sorted(kw.items())))
    if key not in _NC_CACHE:
        _NC_CACHE[key] = build_nc(mm_dt, **kw)
    return _NC_CACHE[key]


def make_in_maps(x, w_attn, b_attn, w_proj, b_proj, mm_dt=MM_DT):
    sd = _np_dt(mm_dt)  # storage dtype for matmul operands
    x = np.asarray(x, dtype=np.float32)
    w_attn = np.asarray(w_attn, dtype=np.float32)
    b_attn = np.asarray(b_attn, dtype=np.float32)
    w_proj = np.asarray(w_proj, dtype=np.float32)
    b_proj = np.asarray(b_proj, dtype=np.float32)

    tri = (np.arange(FI)[None, :] >= np.arange(P)[:, None]).astype(np.float32)
    mask_c = np.concatenate([tri, tri], axis=1)  # duplicated for head pairs
    vones_c = np.ones((P, NHC), dtype=np.float32)

    in_maps = []
    for core in range(8):
        b = core // 4
        hg = core % 4
        sl = slice(JW * hg, JW * (hg + 1))
        in_maps.append({
            "mask_c": mask_c.astype(sd),
            "vones_c": vones_c.astype(sd),
            "xt": np.ascontiguousarray(x[b].T).astype(sd),
            "wq": np.ascontiguousarray(w_attn[:, 0 * C:1 * C][:, sl]).astype(sd),
            "wk": np.ascontiguousarray(w_attn[:, 1 * C:2 * C][:, sl]).astype(sd),
            "wv": np.ascontiguousarray(w_attn[:, 2 * C:3 * C][:, sl]).astype(sd),
            "bq": np.ascontiguousarray(b_attn[0 * C:1 * C][sl]),
            "wp": np.ascontiguousarray(w_proj[sl, :]).astype(sd),
        })
    return in_maps


def _combine(parts, b_attn, w_proj, b_proj):
    parts = [np.asarray(p, dtype=np.float32) for p in parts]
    y0 = parts[0] + parts[1] + parts[2] + parts[3]
    y1 = parts[4] + parts[5] + parts[6] + parts[7]
    bias = (np.asarray(b_attn, np.float32)[2 * C:3 * C] @
            np.asarray(w_proj, np.float32)) + np.asarray(b_proj, np.float32)
    y = np.stack([y0, y1], axis=0) + bias
    return y.astype(np.float32)


def run(x, w_attn, b_attn, w_proj, b_proj, trace=False, mm_dt=MM_DT):
    nc = _get_nc(mm_dt)
    in_maps = make_in_maps(x, w_attn, b_attn, w_proj, b_proj, mm_dt)
    res = run_bass_kernel_spmd(
        nc, in_maps, core_ids=list(range(8)), trace=trace
    )
    parts = [np.asarray(res.results[c]["y"]) for c in range(8)]
    return _combine(parts, b_attn, w_proj, b_proj), res


def kernel(x, w_attn, b_attn, w_proj, b_proj):
    y, _ = run(x, w_attn, b_attn, w_proj, b_proj, trace=False)
    return y


# ---------------------------------------------------------------------------
# Benchmark path: replicates bass2jax.run_bass_via_pjrt's multi-core dispatch
# but WITHOUT donation, so the jitted executable can be re-invoked on
# device-resident buffers to measure steady-state execution wall time.
# ---------------------------------------------------------------------------
def make_bench(x, w_attn, b_attn, w_proj, b_proj, mm_dt=MM_DT, n_cores=8,
               **build_kw):
    import jax
    import concourse.mybir as mb
    from concourse import bass2jax
    from jax.experimental.shard_map import shard_map
    from jax.sharding import Mesh, NamedSharding, PartitionSpec

    nc = _get_nc(mm_dt, **build_kw)
    in_maps = make_in_maps(x, w_attn, b_attn, w_proj, b_proj, mm_dt)
    bass2jax.install_neuronx_cc_hook()

    partition_name = (
        nc.partition_id_tensor.name if nc.partition_id_tensor else None
    )
    in_names, out_names, out_avals, zero_outs = [], [], [], []
    for alloc in nc.m.functions[0].allocations:
        if not isinstance(alloc, mb.MemoryLocationSet):
            continue
        name = alloc.memorylocations[0].name
        if alloc.kind == "ExternalInput":
            if name != partition_name:
                in_names.append(name)
        elif alloc.kind == "ExternalOutput":
            out_names.append(name)
            shape = tuple(alloc.tensor_shape)
            dtype = mb.dt.np(alloc.dtype)
            out_avals.append(jax.core.ShapedArray(shape, dtype))
            zero_outs.append(np.zeros(shape, dtype))
    n_params = len(in_names)
    all_names = in_names + out_names
    if partition_name is not None:
        all_names = all_names + [partition_name]

    def _body(*args):
        operands = list(args)
        if partition_name is not None:
            operands.append(bass2jax.partition_id_tensor())
        outs = bass2jax._bass_exec_p.bind(
            *operands,
            out_avals=tuple(out_avals),
            in_names=tuple(all_names),
            out_names=tuple(out_names),
            lowering_input_output_aliases=(),
            sim_require_finite=True,
            sim_require_nnan=True,
            nc=nc,
        )
        return tuple(outs)

    devices = jax.devices()[:n_cores]
    mesh = Mesh(np.asarray(devices), ("core",))
    spec = PartitionSpec("core")
    f = jax.jit(
        shard_map(
            _body, mesh=mesh,
            in_specs=(spec,) * (n_params + len(out_names)),
            out_specs=(spec,) * len(out_names),
            check_rep=False,
        ),
        keep_unused=True,
    )
    sharding = NamedSharding(mesh, spec)
    args = [
        jax.device_put(
            np.concatenate([np.asarray(m[nm]) for m in in_maps], axis=0),
            sharding,
        )
        for nm in in_names
    ] + [
        jax.device_put(
            np.zeros((n_cores * z.shape[0], *z.shape[1:]), z.dtype), sharding
        )
        for z in zero_outs
    ]
    return f, args, out_names


def bench(x, w_attn, b_attn, w_proj, b_proj, iters=30, mm_dt=MM_DT,
          **build_kw):
    import time

    import jax

    f, args, out_names = make_bench(x, w_attn, b_attn, w_proj, b_proj, mm_dt,
                                    **build_kw)
    out = f(*args)  # compile + warm
    jax.block_until_ready(out)
    times = []
    for _ in range(iters):
        t0 = time.perf_counter()
        out = f(*args)
        jax.block_until_ready(out)
        times.append(time.perf_counter() - t0)
    times.sort()
    y_all = np.asarray(out[out_names.index("y")]).reshape(8, T, C)
    y = _combine([y_all[c] for c in range(8)], b_attn, w_proj, b_proj)
    return y, times



# revision 23
# speedup vs baseline: 1.0115x; 1.0115x over previous
"""Causal self-attention (B=2, T=2048, C=1024, H=16) on 8 TRN2 NeuronCores.

Sharding: core c -> batch b = c // 4, head-group hg = c % 4 (4 heads each).
Each core computes q,k,v for its 4 heads, causal attention, and a partial
output projection (its 256 rows of w_proj). Host sums the 4 partials per
batch.

On-chip layout is fully "transposed" so no on-chip transposes are needed:
  - host passes xT = x[b].T  [C, T]
  - qT, kT computed as [head*64, T] (head-dim on partitions)
  - v computed as [T, head*65] where the 65th column per head is ones
  - scores computed transposed: sT[keys, queries] = kT_h^T-chunk @ qT_h
  - exp on ScalarE (no max subtraction: |logits/8| <= ~8, exp is safe in f32)
  - causal: fully-masked key-chunks skipped; diagonal chunks multiplied by a
    precomputed 0/1 band mask
  - PV matmul lhsT = v_aug[jchunk, head] [128, 65]: rows 0..63 accumulate
    y^T, row 64 (ones) accumulates the softmax denominator -- one matmul
  - normalize: reciprocal of the denominator row, broadcast across 64
    partitions with a stride-0 DMA, multiply on VectorE
  - projection consumes y^T [c_in, t] directly as lhsT

Exact bias simplifications: the k-bias is dropped entirely (a constant
shift along the key axis cancels in softmax), and the v-bias is applied on
the host as y += bv @ w_proj (softmax rows sum to 1), which removes the
per-t-chunk bias matmul from the v chains (-16 matmuls/core).

All matmul operands are bfloat16 (DRAM inputs are converted on the host in
make_in_maps): same PE stream rate as float32r but half the DMA/SBUF
traffic, 2-byte weight loads, and 2x DVE throughput. Measured rel err
3.9e-3 (gate 2e-2). Measured on HW: the kernel's bare 640-matmul stream
takes ~73us; the ScalarE exp stream (~75us incl. per-inst overheads) is the
co-bottleneck, so elementwise work placement, not matmul count, bounds
further gains.
"""

import numpy as np

import concourse.bacc as bacc
import concourse.mybir as mybir
import concourse.tile as tile
from concourse.bass_utils import run_bass_kernel_spmd

P = 128           # partitions
T = 2048          # sequence length
C = 1024          # model dim
NHC = 4           # heads per core
HD = 64           # head dim
JW = NHC * HD     # 256 qkv columns per core
VW = NHC * (HD + 1)  # 260: v + ones column per head
NCC = C // P      # 8 contraction chunks over C
NT = T // P       # 16 key/t chunks of 128
FI = 512          # query chunk (free dim of score matmuls)
NI = T // FI      # 4 query chunks

F32 = mybir.dt.float32
EXPF = mybir.ActivationFunctionType.Exp
COPYF = mybir.ActivationFunctionType.Copy
IDENT = mybir.ActivationFunctionType.Identity

# Delayed-projection placement: which windows' projections are used as PE
# fill work inside which later window's attention loops.
PROJ_SCHED = {2: [0], 3: [1, 2]}

# Matmul input dtype: bfloat16 streams 1 row/cycle on the PE (same rate as
# float32r) but with 2-byte storage: half the DMA/SBUF traffic, a 2-byte
# LDWEIGHTS path (fp32r's 4-byte weight load is ~2x slower and hard to hide
# behind a 512-col stream), and 2x DVE throughput on elementwise ops.
# Accuracy measured on CPU: all-bf16 rel err 3.5e-3 (gate 2e-2).
MM_DT = mybir.dt.bfloat16

# Storage numpy dtype for a given matmul dtype: float32r is a bitcast view of
# f32 bytes; bf16 is a real 2-byte format converted on the host.
def _np_dt(mm_dt):
    if mm_dt in (mybir.dt.float32, mybir.dt.float32r):
        return np.float32
    return mybir.dt.np(mm_dt)


def build_nc(mm_dt=MM_DT, interleave="fine", proj_pool=False,
             mm_bufs=2, s_bufs=2, p_bufs=4, o_bufs=4, reps=1,
             pace=0.75, bcs_q="sp", y_q="sp", xt_g=4,
             split_ss=False, pv_bufs=2, xt0_fine=False, fast_start=True,
             fuse_norm=False, proj_sched=None, qk_dve=False,
             win_order=None, xt0_q="sp", attn_la=2, weights_q="sp",
             win0_split=False, tail_act=True):
    nc = bacc.Bacc(
        "TRN2", target_bir_lowering=False, debug=False, enable_asserts=True
    )

    # DRAM storage dtype for matmul operands: real 2-byte tensors for bf16
    # (host converts in make_in_maps), f32 bytes bitcast for float32/f32r.
    SD = F32 if mm_dt in (F32, mybir.dt.float32r) else mm_dt

    xt_d = nc.dram_tensor("xt", [C, T], SD, kind="ExternalInput")
    wq_d = nc.dram_tensor("wq", [C, JW], SD, kind="ExternalInput")
    wk_d = nc.dram_tensor("wk", [C, JW], SD, kind="ExternalInput")
    wv_d = nc.dram_tensor("wv", [C, JW], SD, kind="ExternalInput")
    bq_d = nc.dram_tensor("bq", [JW], F32, kind="ExternalInput")
    wp_d = nc.dram_tensor("wp", [JW, C], SD, kind="ExternalInput")
    mask_d = nc.dram_tensor("mask_c", [P, 2 * FI], SD, kind="ExternalInput")
    vones_d = nc.dram_tensor("vones_c", [P, NHC], SD, kind="ExternalInput")
    y_d = nc.dram_tensor("y", [T, C], F32, kind="ExternalOutput")

    # Tiles that feed matmuls are declared in the matmul dtype (the BIR
    # verifier requires every producer of an FP32r matmul operand to emit
    # float32r; for bf16 the tiles genuinely store 2-byte values).
    MMD = mm_dt

    def r(ap):  # matmul-operand view of an AP: ensure dtype == mm_dt
        if mm_dt == F32 or ap.dtype == mm_dt:
            return ap
        return ap.bitcast(mm_dt)

    with tile.TileContext(nc) as tc, \
            nc.allow_low_precision(reason="fp32r matmul operand tiles"):
        with (
            tc.tile_pool(name="big", bufs=1) as big,
            tc.tile_pool(name="pp", bufs=p_bufs) as p_pool,
            tc.tile_pool(name="op", bufs=o_bufs) as o_pool,
            tc.tile_pool(name="rows", bufs=2) as row_pool,
            tc.tile_pool(name="psmm", bufs=mm_bufs, space="PSUM") as ps_mm,
            # pair score tiles are [P, 2*FI] = 2 banks each
            tc.tile_pool(name="pss",
                         bufs=s_bufs if s_bufs is not None else 1,
                         space="PSUM") as ps_s,
            tc.tile_pool(name="pspv", bufs=pv_bufs, space="PSUM") as ps_pv,
            tc.tile_pool(name="psbc", bufs=1, space="PSUM") as ps_bc,
            tc.tile_pool(name="pspj", bufs=1, space="PSUM") as ps_pj_real,
        ):
            ps_pj = ps_pj_real if proj_pool else ps_mm
            ps_bcp = ps_bc if proj_pool else ps_mm
            bc_tag = "bc" if proj_pool else "mm"
            # --- constant/weight loads, emitted lazily in compute-gated
            # order (first-window xT and wq first; wp only before proj) ---
            consts = {}
            dma_q = {"sp": nc.sync.dma_start, "gp": nc.gpsimd.dma_start,
                     "dve": nc.vector.dma_start, "act": nc.scalar.dma_start}

            def load_w(nm, d, store, ng=2):
                # ng tiles, NCC/ng contraction chunks each
                cpg = NCC // ng
                store["cpg"] = cpg
                for g in range(ng):
                    load_w_chunk(nm, d, store, g, cpg)

            def load_w_chunk(nm, d, store, g, cpg=1):
                store.setdefault("cpg", cpg)
                wt = big.tile([P, cpg * JW], MMD, tag=f"{nm}{g}",
                              name=f"{nm}{g}")
                ap = d.ap()[cpg * g * P:(cpg * g + cpg) * P, :]
                dma = dma_q[weights_q]
                if cpg == 1:
                    dma(wt[:], r(ap))
                else:
                    dma(
                        wt.rearrange("p (c j) -> p c j", c=cpg),
                        r(ap.rearrange("(c p) j -> p c j", p=P)),
                    )
                store[g] = wt

            def w_sl(store, ci, lo, hi):
                # [P, hi-lo] slice of contraction chunk ci's columns
                cpg = store["cpg"]
                g, c = ci // cpg, ci % cpg
                return store[g][:, c * JW + lo:c * JW + hi]

            def load_biases():
                # bk is dropped exactly (a per-(t,head) constant shift along
                # keys leaves softmax unchanged); bv is folded into the host
                # combine (softmax rows sum to 1 -> y += bv @ w_proj).
                for kc in range(2):
                    bqt = big.tile([P, 1], F32, tag=f"bq{kc}", name=f"bq{kc}")
                    dma_q[weights_q](
                        bqt[:],
                        bq_d.ap()[kc * P:(kc + 1) * P]
                        .rearrange("(p o) -> p o", o=1),
                    )
                    bq_t[kc] = bqt

            def load_consts():
                # maskb[j, u] = 1 if u >= j else 0, duplicated for head pairs
                maskb = big.tile([P, 2 * FI], MMD, tag="maskb", name="maskb")
                dma_q[weights_q](maskb[:], r(mask_d.ap()[:, :]))
                consts["maskb2"] = maskb.rearrange("p (h f) -> p h f", h=2)
                vones = big.tile([P, NHC], MMD, tag="vones", name="vones")
                dma_q[weights_q](vones[:], r(vones_d.ap()[:, :]))
                consts["vones"] = vones

            def load_wp():
                for kc in range(2):
                    wpt = big.tile([P, C], MMD, tag=f"wp{kc}", name=f"wp{kc}")
                    dma_q[weights_q](wpt[:],
                                     r(wp_d.ap()[kc * P:(kc + 1) * P, :]))
                    wp_t[kc] = wpt

            bq_t = {}
            wq_t, wk_t, wv_t, wp_t = {}, {}, {}, {}

            yT = {}
            for kc in range(2):
                for ic in range(NI):
                    yt = big.tile([P, FI], MMD, tag=f"yT{kc}_{ic}",
                                  name=f"yT{kc}_{ic}")
                    yT[(kc, ic)] = yt

            xt_t, qT, kT, v_t = {}, {}, {}, {}

            def xt_groups(ic):
                return 8 if ((xt0_fine or fast_start) and ic == 0) else xt_g

            def emit_xt_chunk(ic, g, cpg):
                xtt = big.tile([P, cpg * FI], MMD, tag=f"xt{g}_{ic}",
                               name=f"xt{g}_{ic}")
                ap = xt_d.ap()[cpg * g * P:(cpg * g + cpg) * P,
                               ic * FI:(ic + 1) * FI]
                dma = dma_q[xt0_q] if ic == 0 else nc.sync.dma_start
                if cpg == 1:
                    dma(xtt[:], r(ap))
                else:
                    dma(
                        xtt.rearrange("p (c u) -> p c u", c=cpg),
                        r(ap.rearrange("(c p) u -> p c u", p=P)),
                    )
                xt_t[(g, ic)] = xtt

            def emit_xt_dma(ic):
                # xT for this t-window: xt_g DMAs, cpg contraction chunks each
                ng = xt_groups(ic)
                cpg = NCC // ng
                for g in range(ng):
                    emit_xt_chunk(ic, g, cpg)

            def xt_sl(ci, ic, lo, hi):
                cpg = NCC // xt_groups(ic)
                g, c = ci // cpg, ci % cpg
                return xt_t[(g, ic)][:, c * FI + lo:c * FI + hi]

            def emit_qk_one(ic, which, kc):
                nm, w_t, store = (("qT", wq_t, qT), ("kT", wk_t, kT))[which]
                ps = ps_mm.tile([P, FI], F32, tag="mm", name="ps_qk")
                for ci in range(NCC):
                    nc.tensor.matmul(
                        ps[:],
                        r(w_sl(w_t, ci, kc * P, (kc + 1) * P)),
                        r(xt_sl(ci, ic, 0, FI)),
                        start=(ci == 0),
                        stop=(ci == NCC - 1),
                    )
                st = big.tile([P, FI], MMD, tag=f"{nm}{kc}_{ic}",
                              name=f"{nm}{kc}_{ic}")
                if qk_dve:
                    # keep ScalarE free for the exp stream: bias-add/copy on
                    # DVE ([P,1] per-partition scalar add)
                    if which == 0:
                        nc.vector.tensor_scalar_add(st[:], ps[:],
                                                    bq_t[kc][:])
                    else:
                        nc.vector.tensor_copy(st[:], ps[:])
                elif which == 0:
                    nc.scalar.activation(st[:], ps[:], IDENT,
                                         bias=bq_t[kc][:], scale=1.0)
                else:
                    nc.scalar.activation(st[:], ps[:], IDENT, scale=1.0)
                store[(kc, ic)] = st

            def emit_qk(ic, which):
                for kc in range(2):
                    emit_qk_one(ic, which, kc)

            def emit_v(ic, half):
                for tc_i in range(4 * ic + 2 * half, 4 * ic + 2 * half + 2):
                    emit_v_one(ic, tc_i)

            def emit_v_one(ic, tc_i):
                if True:
                    ps = ps_mm.tile([P, JW], F32, tag="mm", name="ps_v")
                    for ci in range(NCC):
                        nc.tensor.matmul(
                            ps[:],
                            r(xt_sl(ci, ic, (tc_i % 4) * P,
                                    (tc_i % 4 + 1) * P)),
                            r(w_sl(wv_t, ci, 0, JW)),
                            start=(ci == 0),
                            stop=(ci == NCC - 1),
                        )
                    vt = big.tile([P, VW], MMD, tag=f"v{tc_i}",
                                  name=f"v{tc_i}")
                    vt3 = vt.rearrange("p (h e) -> p h e", e=HD + 1)
                    nc.vector.tensor_copy(
                        vt3[:, :, 0:HD],
                        ps.rearrange("p (h e) -> p h e", e=HD),
                    )
                    nc.vector.tensor_copy(
                        vt3[:, :, HD:HD + 1],
                        consts["vones"].rearrange("p (h o) -> p h o", o=1),
                    )
                    v_t[tc_i] = vt

            def gen_fill_units(qkv_list, proj_list):
                # small PE work units interleaved into attention chunk loops
                for icn in qkv_list:
                    for which in range(2):
                        for kc in range(2):
                            emit_qk_one(icn, which, kc)
                            yield True
                    for tc_i in range(4 * icn, 4 * (icn + 1)):
                        emit_v_one(icn, tc_i)
                        yield True
                for ic_proj in proj_list:
                    for tc_i in range(4 * ic_proj, 4 * (ic_proj + 1)):
                        emit_proj_one(ic_proj, tc_i)
                        yield True

            def emit_qkv_piece(ic, piece):
                if piece == 0:
                    emit_qk(ic, 0)
                elif piece == 1:
                    emit_qk(ic, 1)
                else:
                    emit_v(ic, piece - 2)

            def emit_attention_pair(ic, hp, fill=None, per_cp=0.0,
                                    la=attn_la):
                # attention for query window ic, heads (2*hp, 2*hp+1): both
                # live in partition rows of the kc=hp qT/kT tiles, so their
                # score chunks share one [P, 2*FI] psum tile and ONE exp and
                # mask op each ([P, 2, w] strided APs).
                #
                # Software-pipelined: scores run `la` chunks ahead of the PV
                # matmuls in PE program order, so the in-order PE has score
                # work queued while PV(jc) waits out the exp+mask chain
                # (~1.4us) instead of stalling every chunk.
                kc = hp
                njc = 4 * (ic + 1)
                pv = {}
                for sub in range(2):
                    pv[sub] = ps_pv.tile([HD + 1, FI], F32, tag="pv",
                                         name="ps_pv")

                def emit_score(jc):
                    rr = jc * P - ic * FI  # key offset into query window
                    w = FI - rr if rr > 0 else FI  # valid column suffix
                    pt = p_pool.tile([P, 2 * FI], MMD, tag="p", name="p_t")
                    pt3 = pt.rearrange("p (h f) -> p h f", h=2)
                    ss = ps_s.tile([P, 2 * FI], F32, tag="s", name="ps_s")
                    for sub in range(2):
                        nc.tensor.matmul(
                            ss[:, sub * FI:sub * FI + w],
                            r(kT[(kc, jc // 4)][sub * HD:(sub + 1) * HD,
                                                (jc % 4) * P:
                                                (jc % 4 + 1) * P]),
                            r(qT[(kc, ic)][sub * HD:(sub + 1) * HD,
                                           FI - w:]),
                            start=True,
                            stop=True,
                        )
                    ss3 = ss.rearrange("p (h f) -> p h f", h=2)
                    nc.scalar.activation(pt3[:, :, :w], ss3[:, :, :w],
                                         EXPF, scale=0.125)
                    if rr >= 0:  # diagonal chunk: zero future keys
                        nc.vector.tensor_mul(
                            pt3[:, :, :w], pt3[:, :, :w],
                            consts["maskb2"][:, :, :w]
                        )
                    return pt, w

                def emit_pv(ji, jc, pt, w):
                    for sub in range(2):
                        hh = 2 * hp + sub
                        nc.tensor.matmul(
                            pv[sub][:, FI - w:],
                            r(v_t[jc][:, hh * (HD + 1):(hh + 1) * (HD + 1)]),
                            r(pt[:, sub * FI:sub * FI + w]),
                            start=(ji == 0),
                            stop=(ji == njc - 1),
                            skip_group_check=True,
                        )

                pts = {}
                for jc in range(min(la, njc)):
                    pts[jc] = emit_score(jc)
                credit = 0.0
                for ji, jc in enumerate(range(njc)):
                    if fill is not None:
                        credit += per_cp
                        while credit >= 1.0:
                            credit -= 1.0
                            if next(fill, None) is None:
                                credit = 0.0
                                break
                    if jc + la < njc:
                        pts[jc + la] = emit_score(jc + la)
                    elif la == 0:
                        pts[jc] = emit_score(jc)
                    pt, w = pts.pop(jc)
                    emit_pv(ji, jc, pt, w)
                # broadcast each head's reciprocal row across its 64
                # head-dim partitions with a stride-0-source DMA: keeps the
                # PE and the shared mm psum pool out of the normalize chain.
                # bcs spans all 128 partitions so the SB+SB tensor_mul sees
                # equal base partitions (walrus checkSBSameStartPartition).
                bcs = row_pool.tile([P, FI], F32, tag="bcs", name="bcs")
                for sub in range(2):
                    po = sub * HD
                    rrow = row_pool.tile([1, FI], F32, tag="rr", name="rrow")
                    nc.vector.reciprocal(rrow[:], pv[sub][HD:HD + 1, :])
                    dma_q[bcs_q](
                        bcs[po:po + HD, :],
                        rrow[0:1, :].rearrange("(o b) f -> o b f", b=1)
                        .broadcast_to([1, HD, FI]),
                    )
                    ysl = yT[(kc, ic)][po:po + HD, :]
                    if fuse_norm:
                        # single DVE pass: yT = pv * (1/den) straight from
                        # PSUM (drops the intermediate copy)
                        nc.vector.tensor_mul(ysl, pv[sub][0:HD, :],
                                             bcs[po:po + HD, :])
                    else:
                        nc.vector.tensor_copy(ysl, pv[sub][0:HD, :])
                        nc.vector.tensor_mul(ysl, ysl, bcs[po:po + HD, :])

            def emit_proj(ic):
                # projection for this query window (t chunks 4*ic .. 4*ic+3)
                for tc_i in range(4 * ic, 4 * (ic + 1)):
                    emit_proj_one(ic, tc_i)

            def emit_proj_one(ic, tc_i):
                # one [P, C] output tile and one DMA per t-chunk
                if True:
                    tof = (tc_i % 4) * P
                    ot = o_pool.tile([P, C], F32, tag="o", name="o_t")
                    for n2 in range(2):
                        ps = ps_pj.tile([P, FI], F32,
                                        tag="pj" if proj_pool else "mm",
                                        name="ps_o")
                        for kc in range(2):
                            nc.tensor.matmul(
                                ps[:],
                                r(yT[(kc, ic)][:, tof:tof + P]),
                                r(wp_t[kc][:, n2 * FI:(n2 + 1) * FI]),
                                start=(kc == 0),
                                stop=(kc == 1),
                            )
                        if tail_act and ic == NI - 1 and n2 == 1:
                            # tail window: ScalarE is idle after the last
                            # exp; splitting the drain copies across ACT+DVE
                            # halves the serial tail
                            nc.scalar.activation(
                                ot[:, n2 * FI:(n2 + 1) * FI], ps[:], COPYF)
                        else:
                            nc.vector.tensor_copy(
                                ot[:, n2 * FI:(n2 + 1) * FI], ps[:])
                        if ic == NI - 1:
                            # last window: split the drain so the final DMA
                            # is half-size and starts after the first copy
                            dma_q[y_q](
                                y_d.ap()[tc_i * P:(tc_i + 1) * P,
                                         n2 * FI:(n2 + 1) * FI],
                                ot[:, n2 * FI:(n2 + 1) * FI])
                    if ic != NI - 1:
                        dma_q[y_q](
                            y_d.ap()[tc_i * P:(tc_i + 1) * P, :], ot[:])

            def emit_qkv(ic):
                emit_xt_dma(ic)
                for piece in range(4):
                    emit_qkv_piece(ic, piece)

            def load_front():
                load_biases()
                load_w("wq", wq_d, wq_t)
                load_w("wk", wk_d, wk_t)
                load_w("wv", wv_d, wv_t)
                load_consts()

            for _rep in range(reps):
                if interleave == "fine":
                    # QKV(ic+1) pieces slotted between attention pairs of
                    # window ic: PE fill work while ScalarE runs exp. proj is
                    # delayed one window so the last window (which has no
                    # QKV left) still gets PE fill between its pairs.
                    if _rep == 0:
                        if fast_start:
                            # interleave first-window xt chunks with wq
                            # chunks: the first q chain starts after ~0.5MB
                            # of DMA instead of ~3MB. Biases are tiny and
                            # only needed at the first copy-out, so they go
                            # after the first chunk pair.
                            for ciq in range(NCC):
                                emit_xt_chunk(0, ciq, 1)
                                load_w_chunk("wq", wq_d, wq_t, ciq)
                                if ciq == 0:
                                    load_biases()
                            load_w("wk", wk_d, wk_t)
                            load_w("wv", wv_d, wv_t)
                            load_consts()
                        else:
                            emit_xt_dma(0)
                            load_front()
                        if win0_split:
                            # only what attention(0) hp=0 needs before the
                            # first pair: q/k for kc=0 and all 4 v chunks;
                            # the kc=1 chains become fill inside hp=0
                            emit_qk_one(0, 0, 0)
                            emit_qk_one(0, 1, 0)
                            emit_v(0, 0)
                            emit_v(0, 1)
                        else:
                            for piece in range(4):
                                emit_qkv_piece(0, piece)
                        load_wp()
                    else:
                        emit_qkv(0)
                    # Window schedule: attention order, per-window QKV fill
                    # lists, delayed-projection fill lists, and the tail
                    # projection. "0132" runs the full window 3 before the
                    # shorter window 2, shrinking the un-overlapped tail.
                    if win_order == "0132":
                        sched = [(0, [1], []), (1, [2, 3], []),
                                 (3, [], [0, 1]), (2, [], [3])]
                        tail = [2]
                    else:
                        if proj_sched is None:
                            proj_sched = PROJ_SCHED if NI == 4 else {
                                ic: [ic - 1] for ic in range(1, NI)}
                        sched = [
                            (ic, [ic + 1] if ic + 1 < NI else [],
                             proj_sched.get(ic, []))
                            for ic in range(NI)
                        ]
                        tail = [NI - 1]
                    for wi, (ic, qkv_list, proj_list) in enumerate(sched):
                        for icn in qkv_list:
                            emit_xt_dma(icn)
                        n_units = 8 * len(qkv_list) + 4 * len(proj_list)
                        fill = gen_fill_units(qkv_list, proj_list)
                        prefill = None
                        if win0_split and _rep == 0 and wi == 0:
                            # window-0 kc=1 q/k chains ride as fill in hp=0;
                            # force-drained before hp=1 (which reads them)
                            def _kc1_units():
                                emit_qk_one(0, 0, 1)
                                yield True
                                emit_qk_one(0, 1, 1)
                                yield True
                            prefill = _kc1_units()
                            import itertools
                            fill = itertools.chain(prefill, fill)
                            n_units += 2
                        pc = pace[ic] if isinstance(pace, dict) else pace
                        per_cp = pc * n_units / (2.0 * 4 * (ic + 1))
                        for hp in range(2):
                            emit_attention_pair(ic, hp, fill, per_cp)
                            if hp == 0 and prefill is not None:
                                for _ in prefill:  # ensure kc=1 ready
                                    pass
                        for _ in fill:  # drain leftovers
                            pass
                    for ic_t in tail:
                        emit_proj(ic_t)
                elif interleave:
                    if _rep == 0:
                        emit_xt_dma(0)
                        load_front()
                        load_wp()
                    for ic in range(NI):
                        if _rep == 0 and ic == 0:
                            for piece in range(4):
                                emit_qkv_piece(0, piece)
                        else:
                            emit_qkv(ic)
                        for hp in range(2):
                            emit_attention_pair(ic, hp)
                        emit_proj(ic)
                else:
                    if _rep == 0:
                        emit_xt_dma(0)
                        load_front()
                        load_wp()
                    for ic in range(NI):
                        if _rep == 0 and ic == 0:
                            for piece in range(4):
                                emit_qkv_piece(0, piece)
                        else:
                            emit_qkv(ic)
                    for ic in range(NI):
                        for hp in range(2):
                            emit_attention_pair(ic, hp)
                        emit_proj(ic)



    nc.compile()
    return nc


_NC_CACHE = {}


def _get_nc(mm_dt=MM_DT, **kw):
    key = (str(mm_dt), tuple(sorted(kw.items())))
    if key not in _NC_CACHE:
        _NC_CACHE[key] = build_nc(mm_dt, **kw)
    return _NC_CACHE[key]


def make_in_maps(x, w_attn, b_attn, w_proj, b_proj, mm_dt=MM_DT):
    sd = _np_dt(mm_dt)  # storage dtype for matmul operands
    x = np.asarray(x, dtype=np.float32)
    w_attn = np.asarray(w_attn, dtype=np.float32)
    b_attn = np.asarray(b_attn, dtype=np.float32)
    w_proj = np.asarray(w_proj, dtype=np.float32)
    b_proj = np.asarray(b_proj, dtype=np.float32)

    tri = (np.arange(FI)[None, :] >= np.arange(P)[:, None]).astype(np.float32)
    mask_c = np.concatenate([tri, tri], axis=1)  # duplicated for head pairs
    vones_c = np.ones((P, NHC), dtype=np.float32)

    in_maps = []
    for core in range(8):
        b = core // 4
        hg = core % 4
        sl = slice(JW * hg, JW * (hg + 1))
        in_maps.append({
            "mask_c": mask_c.astype(sd),
            "vones_c": vones_c.astype(sd),
            "xt": np.ascontiguousarray(x[b].T).astype(sd),
            "wq": np.ascontiguousarray(w_attn[:, 0 * C:1 * C][:, sl]).astype(sd),
            "wk": np.ascontiguousarray(w_attn[:, 1 * C:2 * C][:, sl]).astype(sd),
            "wv": np.ascontiguousarray(w_attn[:, 2 * C:3 * C][:, sl]).astype(sd),
            "bq": np.ascontiguousarray(b_attn[0 * C:1 * C][sl]),
            "wp": np.ascontiguousarray(w_proj[sl, :]).astype(sd),
        })
    return in_maps


def _combine(parts, b_attn, w_proj, b_proj):
    parts = [np.asarray(p, dtype=np.float32) for p in parts]
    y0 = parts[0] + parts[1] + parts[2] + parts[3]
    y1 = parts[4] + parts[5] + parts[6] + parts[7]
    bias = (np.asarray(b_attn, np.float32)[2 * C:3 * C] @
            np.asarray(w_proj, np.float32)) + np.asarray(b_proj, np.float32)
    y = np.stack([y0, y1], axis=0) + bias
    return y.astype(np.float32)


def run(x, w_attn, b_attn, w_proj, b_proj, trace=False, mm_dt=MM_DT):
    nc = _get_nc(mm_dt)
    in_maps = make_in_maps(x, w_attn, b_attn, w_proj, b_proj, mm_dt)
    res = run_bass_kernel_spmd(
        nc, in_maps, core_ids=list(range(8)), trace=trace
    )
    parts = [np.asarray(res.results[c]["y"]) for c in range(8)]
    return _combine(parts, b_attn, w_proj, b_proj), res


def kernel(x, w_attn, b_attn, w_proj, b_proj):
    y, _ = run(x, w_attn, b_attn, w_proj, b_proj, trace=False)
    return y


# ---------------------------------------------------------------------------
# Benchmark path: replicates bass2jax.run_bass_via_pjrt's multi-core dispatch
# but WITHOUT donation, so the jitted executable can be re-invoked on
# device-resident buffers to measure steady-state execution wall time.
# ---------------------------------------------------------------------------
def make_bench(x, w_attn, b_attn, w_proj, b_proj, mm_dt=MM_DT, n_cores=8,
               **build_kw):
    import jax
    import concourse.mybir as mb
    from concourse import bass2jax
    from jax.experimental.shard_map import shard_map
    from jax.sharding import Mesh, NamedSharding, PartitionSpec

    nc = _get_nc(mm_dt, **build_kw)
    in_maps = make_in_maps(x, w_attn, b_attn, w_proj, b_proj, mm_dt)
    bass2jax.install_neuronx_cc_hook()

    partition_name = (
        nc.partition_id_tensor.name if nc.partition_id_tensor else None
    )
    in_names, out_names, out_avals, zero_outs = [], [], [], []
    for alloc in nc.m.functions[0].allocations:
        if not isinstance(alloc, mb.MemoryLocationSet):
            continue
        name = alloc.memorylocations[0].name
        if alloc.kind == "ExternalInput":
            if name != partition_name:
                in_names.append(name)
        elif alloc.kind == "ExternalOutput":
            out_names.append(name)
            shape = tuple(alloc.tensor_shape)
            dtype = mb.dt.np(alloc.dtype)
            out_avals.append(jax.core.ShapedArray(shape, dtype))
            zero_outs.append(np.zeros(shape, dtype))
    n_params = len(in_names)
    all_names = in_names + out_names
    if partition_name is not None:
        all_names = all_names + [partition_name]

    def _body(*args):
        operands = list(args)
        if partition_name is not None:
            operands.append(bass2jax.partition_id_tensor())
        outs = bass2jax._bass_exec_p.bind(
            *operands,
            out_avals=tuple(out_avals),
            in_names=tuple(all_names),
            out_names=tuple(out_names),
            lowering_input_output_aliases=(),
            sim_require_finite=True,
            sim_require_nnan=True,
            nc=nc,
        )
        return tuple(outs)

    devices = jax.devices()[:n_cores]
    mesh = Mesh(np.asarray(devices), ("core",))
    spec = PartitionSpec("core")
    f = jax.jit(
        shard_map(
            _body, mesh=mesh,
            in_specs=(spec,) * (n_params + len(out_names)),
            out_specs=(spec,) * len(out_names),
            check_rep=False,
        ),
        keep_unused=True,
    )
    sharding = NamedSharding(mesh, spec)
    args = [
        jax.device_put(
            np.concatenate([np.asarray(m[nm]) for m in in_maps], axis=0),
            sharding,
        )
        for nm in in_names
    ] + [
        jax.device_put(
            np.zeros((n_cores * z.shape[0], *z.shape[1:]), z.dtype), sharding
        )
        for z in zero_outs
    ]
    return f, args, out_names


def bench(x, w_attn, b_attn, w_proj, b_proj, iters=30, mm_dt=MM_DT,
          **build_kw):
    import time

    import jax

    f, args, out_names = make_bench(x, w_attn, b_attn, w_proj, b_proj, mm_dt,
                                    **build_kw)
    out = f(*args)  # compile + warm
    jax.block_until_ready(out)
    times = []
    for _ in range(iters):
        t0 = time.perf_counter()
        out = f(*args)
        jax.block_until_ready(out)
        times.append(time.perf_counter() - t0)
    times.sort()
    y_all = np.asarray(out[out_names.index("y")]).reshape(8, T, C)
    y = _combine([y_all[c] for c in range(8)], b_attn, w_proj, b_proj)
    return y, times



# revision 26
# speedup vs baseline: 1.1028x; 1.0903x over previous
"""Causal self-attention (B=2, T=2048, C=1024, H=16) on 8 TRN2 NeuronCores.

Sharding: core c -> batch b = c // 4, head-group hg = c % 4 (4 heads each).
Each core computes q,k,v for its 4 heads, causal attention, and a partial
output projection (its 256 rows of w_proj). Host sums the 4 partials per
batch.

On-chip layout is fully "transposed" so no on-chip transposes are needed:
  - host passes xT = x[b].T  [C, T]
  - qT, kT computed as [head*64, T] (head-dim on partitions)
  - v computed as [T, head*65] where the 65th column per head is ones
  - scores computed transposed: sT[keys, queries] = kT_h^T-chunk @ qT_h
  - exp on ScalarE (no max subtraction: |logits/8| <= ~8, exp is safe in f32)
  - causal: fully-masked key-chunks skipped; diagonal chunks multiplied by a
    precomputed 0/1 band mask
  - PV matmul lhsT = v_aug[jchunk, head] [128, 65]: rows 0..63 accumulate
    y^T, row 64 (ones) accumulates the softmax denominator -- one matmul
  - normalize: reciprocal of the denominator row, broadcast across 64
    partitions with a stride-0 DMA, multiply on VectorE
  - projection consumes y^T [c_in, t] directly as lhsT

Exact bias simplifications: the k-bias is dropped entirely (a constant
shift along the key axis cancels in softmax), and the v-bias is applied on
the host as y += bv @ w_proj (softmax rows sum to 1), which removes the
per-t-chunk bias matmul from the v chains (-16 matmuls/core).

All matmul operands are bfloat16 (DRAM inputs are converted on the host in
make_in_maps): same PE stream rate as float32r but half the DMA/SBUF
traffic, 2-byte weight loads, and 2x DVE throughput. Measured rel err
3.9e-3 (gate 2e-2). Measured on HW: the kernel's bare 640-matmul stream
takes ~73us; the ScalarE exp stream (~75us incl. per-inst overheads) is the
co-bottleneck, so elementwise work placement, not matmul count, bounds
further gains.
"""

import numpy as np

import concourse.bacc as bacc
import concourse.mybir as mybir
import concourse.tile as tile
from concourse.bass_utils import run_bass_kernel_spmd

P = 128           # partitions
T = 2048          # sequence length
C = 1024          # model dim
NHC = 4           # heads per core
HD = 64           # head dim
JW = NHC * HD     # 256 qkv columns per core
VW = NHC * (HD + 1)  # 260: v + ones column per head
NCC = C // P      # 8 contraction chunks over C
NT = T // P       # 16 key/t chunks of 128
FI = 512          # query chunk (free dim of score matmuls)
NI = T // FI      # 4 query chunks

F32 = mybir.dt.float32
EXPF = mybir.ActivationFunctionType.Exp
COPYF = mybir.ActivationFunctionType.Copy
IDENT = mybir.ActivationFunctionType.Identity

# Delayed-projection placement: which windows' projections are used as PE
# fill work inside which later window's attention loops.
PROJ_SCHED = {2: [0], 3: [1, 2]}

# Matmul input dtype: bfloat16 streams 1 row/cycle on the PE (same rate as
# float32r) but with 2-byte storage: half the DMA/SBUF traffic, a 2-byte
# LDWEIGHTS path (fp32r's 4-byte weight load is ~2x slower and hard to hide
# behind a 512-col stream), and 2x DVE throughput on elementwise ops.
# Accuracy measured on CPU: all-bf16 rel err 3.5e-3 (gate 2e-2).
MM_DT = mybir.dt.bfloat16

# Storage numpy dtype for a given matmul dtype: float32r is a bitcast view of
# f32 bytes; bf16 is a real 2-byte format converted on the host.
def _np_dt(mm_dt):
    if mm_dt in (mybir.dt.float32, mybir.dt.float32r):
        return np.float32
    return mybir.dt.np(mm_dt)


def build_nc(mm_dt=MM_DT, interleave="fine", proj_pool=False,
             mm_bufs=2, s_bufs=2, p_bufs=4, o_bufs=4, reps=1,
             pace=0.75, bcs_q="sp", y_q="sp", xt_g=4,
             split_ss=False, pv_bufs=2, xt0_fine=False, fast_start=True,
             fuse_norm=False, proj_sched=None, qk_dve=False,
             win_order=None, xt0_q="sp", attn_la=2, weights_q="sp",
             win0_split=False, tail_act=True, dve_exp=False,
             schrau_c=486411):
    nc = bacc.Bacc(
        "TRN2", target_bir_lowering=False, debug=False, enable_asserts=True
    )

    # DRAM storage dtype for matmul operands: real 2-byte tensors for bf16
    # (host converts in make_in_maps), f32 bytes bitcast for float32/f32r.
    SD = F32 if mm_dt in (F32, mybir.dt.float32r) else mm_dt

    xt_d = nc.dram_tensor("xt", [C, T], SD, kind="ExternalInput")
    wq_d = nc.dram_tensor("wq", [C, JW], SD, kind="ExternalInput")
    wk_d = nc.dram_tensor("wk", [C, JW], SD, kind="ExternalInput")
    wv_d = nc.dram_tensor("wv", [C, JW], SD, kind="ExternalInput")
    bq_d = nc.dram_tensor("bq", [JW], F32, kind="ExternalInput")
    wp_d = nc.dram_tensor("wp", [JW, C], SD, kind="ExternalInput")
    mask_d = nc.dram_tensor("mask_c", [P, 2 * FI], SD, kind="ExternalInput")
    vones_d = nc.dram_tensor("vones_c", [P, NHC], SD, kind="ExternalInput")
    y_d = nc.dram_tensor("y", [T, C], F32, kind="ExternalOutput")

    # Tiles that feed matmuls are declared in the matmul dtype (the BIR
    # verifier requires every producer of an FP32r matmul operand to emit
    # float32r; for bf16 the tiles genuinely store 2-byte values).
    MMD = mm_dt

    def r(ap):  # matmul-operand view of an AP: ensure dtype == mm_dt
        if mm_dt == F32 or ap.dtype == mm_dt:
            return ap
        return ap.bitcast(mm_dt)

    with tile.TileContext(nc) as tc, \
            nc.allow_low_precision(reason="fp32r matmul operand tiles"):
        with (
            tc.tile_pool(name="big", bufs=1) as big,
            tc.tile_pool(name="pp", bufs=p_bufs) as p_pool,
            tc.tile_pool(name="ip", bufs=2) as i_pool,
            tc.tile_pool(name="op", bufs=o_bufs) as o_pool,
            tc.tile_pool(name="rows", bufs=2) as row_pool,
            tc.tile_pool(name="psmm", bufs=mm_bufs, space="PSUM") as ps_mm,
            # pair score tiles are [P, 2*FI] = 2 banks each
            tc.tile_pool(name="pss",
                         bufs=s_bufs if s_bufs is not None else 1,
                         space="PSUM") as ps_s,
            tc.tile_pool(name="pspv", bufs=pv_bufs, space="PSUM") as ps_pv,
            tc.tile_pool(name="psbc", bufs=1, space="PSUM") as ps_bc,
            tc.tile_pool(name="pspj", bufs=1, space="PSUM") as ps_pj_real,
        ):
            ps_pj = ps_pj_real if proj_pool else ps_mm
            ps_bcp = ps_bc if proj_pool else ps_mm
            bc_tag = "bc" if proj_pool else "mm"
            # --- constant/weight loads, emitted lazily in compute-gated
            # order (first-window xT and wq first; wp only before proj) ---
            consts = {}
            dma_q = {"sp": nc.sync.dma_start, "gp": nc.gpsimd.dma_start,
                     "dve": nc.vector.dma_start, "act": nc.scalar.dma_start}

            def load_w(nm, d, store, ng=2):
                # ng tiles, NCC/ng contraction chunks each
                cpg = NCC // ng
                store["cpg"] = cpg
                for g in range(ng):
                    load_w_chunk(nm, d, store, g, cpg)

            def load_w_chunk(nm, d, store, g, cpg=1):
                store.setdefault("cpg", cpg)
                wt = big.tile([P, cpg * JW], MMD, tag=f"{nm}{g}",
                              name=f"{nm}{g}")
                ap = d.ap()[cpg * g * P:(cpg * g + cpg) * P, :]
                dma = dma_q[weights_q]
                if cpg == 1:
                    dma(wt[:], r(ap))
                else:
                    dma(
                        wt.rearrange("p (c j) -> p c j", c=cpg),
                        r(ap.rearrange("(c p) j -> p c j", p=P)),
                    )
                store[g] = wt

            def w_sl(store, ci, lo, hi):
                # [P, hi-lo] slice of contraction chunk ci's columns
                cpg = store["cpg"]
                g, c = ci // cpg, ci % cpg
                return store[g][:, c * JW + lo:c * JW + hi]

            def load_biases():
                # bk is dropped exactly (a per-(t,head) constant shift along
                # keys leaves softmax unchanged); bv is folded into the host
                # combine (softmax rows sum to 1 -> y += bv @ w_proj).
                for kc in range(2):
                    bqt = big.tile([P, 1], F32, tag=f"bq{kc}", name=f"bq{kc}")
                    dma_q[weights_q](
                        bqt[:],
                        bq_d.ap()[kc * P:(kc + 1) * P]
                        .rearrange("(p o) -> p o", o=1),
                    )
                    bq_t[kc] = bqt

            def load_consts():
                # maskb[j, u] = 1 if u >= j else 0, duplicated for head pairs
                maskb = big.tile([P, 2 * FI], MMD, tag="maskb", name="maskb")
                dma_q[weights_q](maskb[:], r(mask_d.ap()[:, :]))
                consts["maskb2"] = maskb.rearrange("p (h f) -> p h f", h=2)
                vones = big.tile([P, NHC], MMD, tag="vones", name="vones")
                dma_q[weights_q](vones[:], r(vones_d.ap()[:, :]))
                consts["vones"] = vones

            def load_wp():
                for kc in range(2):
                    wpt = big.tile([P, C], MMD, tag=f"wp{kc}", name=f"wp{kc}")
                    dma_q[weights_q](wpt[:],
                                     r(wp_d.ap()[kc * P:(kc + 1) * P, :]))
                    wp_t[kc] = wpt

            bq_t = {}
            wq_t, wk_t, wv_t, wp_t = {}, {}, {}, {}

            yT = {}
            for kc in range(2):
                for ic in range(NI):
                    yt = big.tile([P, FI], MMD, tag=f"yT{kc}_{ic}",
                                  name=f"yT{kc}_{ic}")
                    yT[(kc, ic)] = yt

            xt_t, qT, kT, v_t = {}, {}, {}, {}

            def xt_groups(ic):
                return 8 if ((xt0_fine or fast_start) and ic == 0) else xt_g

            def emit_xt_chunk(ic, g, cpg):
                xtt = big.tile([P, cpg * FI], MMD, tag=f"xt{g}_{ic}",
                               name=f"xt{g}_{ic}")
                ap = xt_d.ap()[cpg * g * P:(cpg * g + cpg) * P,
                               ic * FI:(ic + 1) * FI]
                dma = dma_q[xt0_q] if ic == 0 else nc.sync.dma_start
                if cpg == 1:
                    dma(xtt[:], r(ap))
                else:
                    dma(
                        xtt.rearrange("p (c u) -> p c u", c=cpg),
                        r(ap.rearrange("(c p) u -> p c u", p=P)),
                    )
                xt_t[(g, ic)] = xtt

            def emit_xt_dma(ic):
                # xT for this t-window: xt_g DMAs, cpg contraction chunks each
                ng = xt_groups(ic)
                cpg = NCC // ng
                for g in range(ng):
                    emit_xt_chunk(ic, g, cpg)

            def xt_sl(ci, ic, lo, hi):
                cpg = NCC // xt_groups(ic)
                g, c = ci // cpg, ci % cpg
                return xt_t[(g, ic)][:, c * FI + lo:c * FI + hi]

            def emit_qk_one(ic, which, kc):
                nm, w_t, store = (("qT", wq_t, qT), ("kT", wk_t, kT))[which]
                ps = ps_mm.tile([P, FI], F32, tag="mm", name="ps_qk")
                for ci in range(NCC):
                    nc.tensor.matmul(
                        ps[:],
                        r(w_sl(w_t, ci, kc * P, (kc + 1) * P)),
                        r(xt_sl(ci, ic, 0, FI)),
                        start=(ci == 0),
                        stop=(ci == NCC - 1),
                    )
                st = big.tile([P, FI], MMD, tag=f"{nm}{kc}_{ic}",
                              name=f"{nm}{kc}_{ic}")
                if qk_dve:
                    # keep ScalarE free for the exp stream: bias-add/copy on
                    # DVE ([P,1] per-partition scalar add)
                    if which == 0:
                        nc.vector.tensor_scalar_add(st[:], ps[:],
                                                    bq_t[kc][:])
                    else:
                        nc.vector.tensor_copy(st[:], ps[:])
                elif which == 0:
                    nc.scalar.activation(st[:], ps[:], IDENT,
                                         bias=bq_t[kc][:], scale=1.0)
                else:
                    nc.scalar.activation(st[:], ps[:], IDENT, scale=1.0)
                store[(kc, ic)] = st

            def emit_qk(ic, which):
                for kc in range(2):
                    emit_qk_one(ic, which, kc)

            def emit_v(ic, half):
                for tc_i in range(4 * ic + 2 * half, 4 * ic + 2 * half + 2):
                    emit_v_one(ic, tc_i)

            def emit_v_one(ic, tc_i):
                if True:
                    ps = ps_mm.tile([P, JW], F32, tag="mm", name="ps_v")
                    for ci in range(NCC):
                        nc.tensor.matmul(
                            ps[:],
                            r(xt_sl(ci, ic, (tc_i % 4) * P,
                                    (tc_i % 4 + 1) * P)),
                            r(w_sl(wv_t, ci, 0, JW)),
                            start=(ci == 0),
                            stop=(ci == NCC - 1),
                        )
                    vt = big.tile([P, VW], MMD, tag=f"v{tc_i}",
                                  name=f"v{tc_i}")
                    vt3 = vt.rearrange("p (h e) -> p h e", e=HD + 1)
                    nc.vector.tensor_copy(
                        vt3[:, :, 0:HD],
                        ps.rearrange("p (h e) -> p h e", e=HD),
                    )
                    nc.vector.tensor_copy(
                        vt3[:, :, HD:HD + 1],
                        consts["vones"].rearrange("p (h o) -> p h o", o=1),
                    )
                    v_t[tc_i] = vt

            def gen_fill_units(qkv_list, proj_list):
                # small PE work units interleaved into attention chunk loops
                for icn in qkv_list:
                    for which in range(2):
                        for kc in range(2):
                            emit_qk_one(icn, which, kc)
                            yield True
                    for tc_i in range(4 * icn, 4 * (icn + 1)):
                        emit_v_one(icn, tc_i)
                        yield True
                for ic_proj in proj_list:
                    for tc_i in range(4 * ic_proj, 4 * (ic_proj + 1)):
                        emit_proj_one(ic_proj, tc_i)
                        yield True

            def emit_qkv_piece(ic, piece):
                if piece == 0:
                    emit_qk(ic, 0)
                elif piece == 1:
                    emit_qk(ic, 1)
                else:
                    emit_v(ic, piece - 2)

            def emit_attention_pair(ic, hp, fill=None, per_cp=0.0,
                                    la=attn_la):
                # attention for query window ic, heads (2*hp, 2*hp+1): both
                # live in partition rows of the kc=hp qT/kT tiles, so their
                # score chunks share one [P, 2*FI] psum tile and ONE exp and
                # mask op each ([P, 2, w] strided APs).
                #
                # Software-pipelined: scores run `la` chunks ahead of the PV
                # matmuls in PE program order, so the in-order PE has score
                # work queued while PV(jc) waits out the exp+mask chain
                # (~1.4us) instead of stalling every chunk.
                kc = hp
                njc = 4 * (ic + 1)
                pv = {}
                for sub in range(2):
                    pv[sub] = ps_pv.tile([HD + 1, FI], F32, tag="pv",
                                         name="ps_pv")

                def emit_score(jc):
                    rr = jc * P - ic * FI  # key offset into query window
                    w = FI - rr if rr > 0 else FI  # valid column suffix
                    pt = p_pool.tile([P, 2 * FI], MMD, tag="p", name="p_t")
                    pt3 = pt.rearrange("p (h f) -> p h f", h=2)
                    ss = ps_s.tile([P, 2 * FI], F32, tag="s", name="ps_s")
                    for sub in range(2):
                        nc.tensor.matmul(
                            ss[:, sub * FI:sub * FI + w],
                            r(kT[(kc, jc // 4)][sub * HD:(sub + 1) * HD,
                                                (jc % 4) * P:
                                                (jc % 4 + 1) * P]),
                            r(qT[(kc, ic)][sub * HD:(sub + 1) * HD,
                                           FI - w:]),
                            start=True,
                            stop=True,
                        )
                    ss3 = ss.rearrange("p (h f) -> p h f", h=2)
                    if dve_exp and rr < 0 and jc % 4 == 1:
                        # Schraudolph exp on DVE for a slice of the full
                        # chunks: p = bitcast_f32(int32(A*0.125*s + B)).
                        # Relieves the ScalarE exp ceiling; rel err ~1.5%
                        # rms on the routed ~15% of p values, which largely
                        # cancels between softmax numerator and denominator.
                        a = (2.0 ** 23 / np.log(2.0)) * 0.125
                        b = float(1065353216 - schrau_c)
                        it = i_pool.tile([P, 2 * FI], mybir.dt.int32,
                                         tag="i", name="i_t")
                        nc.vector.tensor_scalar(
                            it[:], ss[:], a, b,
                            mybir.AluOpType.mult, mybir.AluOpType.add)
                        nc.vector.tensor_copy(pt[:], it[:].bitcast(F32))
                    else:
                        nc.scalar.activation(pt3[:, :, :w], ss3[:, :, :w],
                                             EXPF, scale=0.125)
                        if rr >= 0:  # diagonal chunk: zero future keys
                            nc.vector.tensor_mul(
                                pt3[:, :, :w], pt3[:, :, :w],
                                consts["maskb2"][:, :, :w]
                            )
                    return pt, w

                def emit_pv(ji, jc, pt, w):
                    for sub in range(2):
                        hh = 2 * hp + sub
                        nc.tensor.matmul(
                            pv[sub][:, FI - w:],
                            r(v_t[jc][:, hh * (HD + 1):(hh + 1) * (HD + 1)]),
                            r(pt[:, sub * FI:sub * FI + w]),
                            start=(ji == 0),
                            stop=(ji == njc - 1),
                            skip_group_check=True,
                        )

                pts = {}
                for jc in range(min(la, njc)):
                    pts[jc] = emit_score(jc)
                credit = 0.0
                for ji, jc in enumerate(range(njc)):
                    if fill is not None:
                        credit += per_cp
                        while credit >= 1.0:
                            credit -= 1.0
                            if next(fill, None) is None:
                                credit = 0.0
                                break
                    if jc + la < njc:
                        pts[jc + la] = emit_score(jc + la)
                    elif la == 0:
                        pts[jc] = emit_score(jc)
                    pt, w = pts.pop(jc)
                    emit_pv(ji, jc, pt, w)
                # broadcast each head's reciprocal row across its 64
                # head-dim partitions with a stride-0-source DMA: keeps the
                # PE and the shared mm psum pool out of the normalize chain.
                # bcs spans all 128 partitions so the SB+SB tensor_mul sees
                # equal base partitions (walrus checkSBSameStartPartition).
                bcs = row_pool.tile([P, FI], F32, tag="bcs", name="bcs")
                for sub in range(2):
                    po = sub * HD
                    rrow = row_pool.tile([1, FI], F32, tag="rr", name="rrow")
                    nc.vector.reciprocal(rrow[:], pv[sub][HD:HD + 1, :])
                    dma_q[bcs_q](
                        bcs[po:po + HD, :],
                        rrow[0:1, :].rearrange("(o b) f -> o b f", b=1)
                        .broadcast_to([1, HD, FI]),
                    )
                    ysl = yT[(kc, ic)][po:po + HD, :]
                    if fuse_norm:
                        # single DVE pass: yT = pv * (1/den) straight from
                        # PSUM (drops the intermediate copy)
                        nc.vector.tensor_mul(ysl, pv[sub][0:HD, :],
                                             bcs[po:po + HD, :])
                    else:
                        nc.vector.tensor_copy(ysl, pv[sub][0:HD, :])
                        nc.vector.tensor_mul(ysl, ysl, bcs[po:po + HD, :])

            def emit_proj(ic):
                # projection for this query window (t chunks 4*ic .. 4*ic+3)
                for tc_i in range(4 * ic, 4 * (ic + 1)):
                    emit_proj_one(ic, tc_i)

            def emit_proj_one(ic, tc_i):
                # one [P, C] output tile and one DMA per t-chunk
                if True:
                    tof = (tc_i % 4) * P
                    ot = o_pool.tile([P, C], F32, tag="o", name="o_t")
                    for n2 in range(2):
                        ps = ps_pj.tile([P, FI], F32,
                                        tag="pj" if proj_pool else "mm",
                                        name="ps_o")
                        for kc in range(2):
                            nc.tensor.matmul(
                                ps[:],
                                r(yT[(kc, ic)][:, tof:tof + P]),
                                r(wp_t[kc][:, n2 * FI:(n2 + 1) * FI]),
                                start=(kc == 0),
                                stop=(kc == 1),
                            )
                        if tail_act and ic == NI - 1 and n2 == 1:
                            # tail window: ScalarE is idle after the last
                            # exp; splitting the drain copies across ACT+DVE
                            # halves the serial tail
                            nc.scalar.activation(
                                ot[:, n2 * FI:(n2 + 1) * FI], ps[:], COPYF)
                        else:
                            nc.vector.tensor_copy(
                                ot[:, n2 * FI:(n2 + 1) * FI], ps[:])
                        if ic == NI - 1:
                            # last window: split the drain so the final DMA
                            # is half-size and starts after the first copy
                            dma_q[y_q](
                                y_d.ap()[tc_i * P:(tc_i + 1) * P,
                                         n2 * FI:(n2 + 1) * FI],
                                ot[:, n2 * FI:(n2 + 1) * FI])
                    if ic != NI - 1:
                        dma_q[y_q](
                            y_d.ap()[tc_i * P:(tc_i + 1) * P, :], ot[:])

            def emit_qkv(ic):
                emit_xt_dma(ic)
                for piece in range(4):
                    emit_qkv_piece(ic, piece)

            def load_front():
                load_biases()
                load_w("wq", wq_d, wq_t)
                load_w("wk", wk_d, wk_t)
                load_w("wv", wv_d, wv_t)
                load_consts()

            for _rep in range(reps):
                if interleave == "fine":
                    # QKV(ic+1) pieces slotted between attention pairs of
                    # window ic: PE fill work while ScalarE runs exp. proj is
                    # delayed one window so the last window (which has no
                    # QKV left) still gets PE fill between its pairs.
                    if _rep == 0:
                        if fast_start:
                            # interleave first-window xt chunks with wq
                            # chunks: the first q chain starts after ~0.5MB
                            # of DMA instead of ~3MB. Biases are tiny and
                            # only needed at the first copy-out, so they go
                            # after the first chunk pair.
                            for ciq in range(NCC):
                                emit_xt_chunk(0, ciq, 1)
                                load_w_chunk("wq", wq_d, wq_t, ciq)
                                if ciq == 0:
                                    load_biases()
                            load_w("wk", wk_d, wk_t)
                            load_w("wv", wv_d, wv_t)
                            load_consts()
                        else:
                            emit_xt_dma(0)
                            load_front()
                        if win0_split:
                            # only what attention(0) hp=0 needs before the
                            # first pair: q/k for kc=0 and all 4 v chunks;
                            # the kc=1 chains become fill inside hp=0
                            emit_qk_one(0, 0, 0)
                            emit_qk_one(0, 1, 0)
                            emit_v(0, 0)
                            emit_v(0, 1)
                        else:
                            for piece in range(4):
                                emit_qkv_piece(0, piece)
                        load_wp()
                    else:
                        emit_qkv(0)
                    # Window schedule: attention order, per-window QKV fill
                    # lists, delayed-projection fill lists, and the tail
                    # projection. "0132" runs the full window 3 before the
                    # shorter window 2, shrinking the un-overlapped tail.
                    if win_order == "0132":
                        sched = [(0, [1], []), (1, [2, 3], []),
                                 (3, [], [0, 1]), (2, [], [3])]
                        tail = [2]
                    else:
                        if proj_sched is None:
                            proj_sched = PROJ_SCHED if NI == 4 else {
                                ic: [ic - 1] for ic in range(1, NI)}
                        sched = [
                            (ic, [ic + 1] if ic + 1 < NI else [],
                             proj_sched.get(ic, []))
                            for ic in range(NI)
                        ]
                        tail = [NI - 1]
                    for wi, (ic, qkv_list, proj_list) in enumerate(sched):
                        for icn in qkv_list:
                            emit_xt_dma(icn)
                        n_units = 8 * len(qkv_list) + 4 * len(proj_list)
                        fill = gen_fill_units(qkv_list, proj_list)
                        prefill = None
                        if win0_split and _rep == 0 and wi == 0:
                            # window-0 kc=1 q/k chains ride as fill in hp=0;
                            # force-drained before hp=1 (which reads them)
                            def _kc1_units():
                                emit_qk_one(0, 0, 1)
                                yield True
                                emit_qk_one(0, 1, 1)
                                yield True
                            prefill = _kc1_units()
                            import itertools
                            fill = itertools.chain(prefill, fill)
                            n_units += 2
                        pc = pace[ic] if isinstance(pace, dict) else pace
                        per_cp = pc * n_units / (2.0 * 4 * (ic + 1))
                        for hp in range(2):
                            emit_attention_pair(ic, hp, fill, per_cp)
                            if hp == 0 and prefill is not None:
                                for _ in prefill:  # ensure kc=1 ready
                                    pass
                        for _ in fill:  # drain leftovers
                            pass
                    for ic_t in tail:
                        emit_proj(ic_t)
                elif interleave:
                    if _rep == 0:
                        emit_xt_dma(0)
                        load_front()
                        load_wp()
                    for ic in range(NI):
                        if _rep == 0 and ic == 0:
                            for piece in range(4):
                                emit_qkv_piece(0, piece)
                        else:
                            emit_qkv(ic)
                        for hp in range(2):
                            emit_attention_pair(ic, hp)
                        emit_proj(ic)
                else:
                    if _rep == 0:
                        emit_xt_dma(0)
                        load_front()
                        load_wp()
                    for ic in range(NI):
                        if _rep == 0 and ic == 0:
                            for piece in range(4):
                                emit_qkv_piece(0, piece)
                        else:
                            emit_qkv(ic)
                    for ic in range(NI):
                        for hp in range(2):
                            emit_attention_pair(ic, hp)
                        emit_proj(ic)



    nc.compile()
    return nc


_NC_CACHE = {}


def _get_nc(mm_dt=MM_DT, **kw):
    key = (str(mm_dt), tuple(sorted(kw.items())))
    if key not in _NC_CACHE:
        _NC_CACHE[key] = build_nc(mm_dt, **kw)
    return _NC_CACHE[key]


def make_in_maps(x, w_attn, b_attn, w_proj, b_proj, mm_dt=MM_DT):
    sd = _np_dt(mm_dt)  # storage dtype for matmul operands
    x = np.asarray(x, dtype=np.float32)
    w_attn = np.asarray(w_attn, dtype=np.float32)
    b_attn = np.asarray(b_attn, dtype=np.float32)
    w_proj = np.asarray(w_proj, dtype=np.float32)
    b_proj = np.asarray(b_proj, dtype=np.float32)

    tri = (np.arange(FI)[None, :] >= np.arange(P)[:, None]).astype(np.float32)
    mask_c = np.concatenate([tri, tri], axis=1)  # duplicated for head pairs
    vones_c = np.ones((P, NHC), dtype=np.float32)

    in_maps = []
    for core in range(8):
        b = core // 4
        hg = core % 4
        sl = slice(JW * hg, JW * (hg + 1))
        in_maps.append({
            "mask_c": mask_c.astype(sd),
            "vones_c": vones_c.astype(sd),
            "xt": np.ascontiguousarray(x[b].T).astype(sd),
            "wq": np.ascontiguousarray(w_attn[:, 0 * C:1 * C][:, sl]).astype(sd),
            "wk": np.ascontiguousarray(w_attn[:, 1 * C:2 * C][:, sl]).astype(sd),
            "wv": np.ascontiguousarray(w_attn[:, 2 * C:3 * C][:, sl]).astype(sd),
            "bq": np.ascontiguousarray(b_attn[0 * C:1 * C][sl]),
            "wp": np.ascontiguousarray(w_proj[sl, :]).astype(sd),
        })
    return in_maps


def _combine(parts, b_attn, w_proj, b_proj):
    parts = [np.asarray(p, dtype=np.float32) for p in parts]
    y0 = parts[0] + parts[1] + parts[2] + parts[3]
    y1 = parts[4] + parts[5] + parts[6] + parts[7]
    bias = (np.asarray(b_attn, np.float32)[2 * C:3 * C] @
            np.asarray(w_proj, np.float32)) + np.asarray(b_proj, np.float32)
    y = np.stack([y0, y1], axis=0) + bias
    return y.astype(np.float32)


def run(x, w_attn, b_attn, w_proj, b_proj, trace=False, mm_dt=MM_DT):
    nc = _get_nc(mm_dt)
    in_maps = make_in_maps(x, w_attn, b_attn, w_proj, b_proj, mm_dt)
    res = run_bass_kernel_spmd(
        nc, in_maps, core_ids=list(range(8)), trace=trace
    )
    parts = [np.asarray(res.results[c]["y"]) for c in range(8)]
    return _combine(parts, b_attn, w_proj, b_proj), res


def kernel(x, w_attn, b_attn, w_proj, b_proj):
    y, _ = run(x, w_attn, b_attn, w_proj, b_proj, trace=False)
    return y


# ---------------------------------------------------------------------------
# Benchmark path: replicates bass2jax.run_bass_via_pjrt's multi-core dispatch
# but WITHOUT donation, so the jitted executable can be re-invoked on
# device-resident buffers to measure steady-state execution wall time.
# ---------------------------------------------------------------------------
def make_bench(x, w_attn, b_attn, w_proj, b_proj, mm_dt=MM_DT, n_cores=8,
               **build_kw):
    import jax
    import concourse.mybir as mb
    from concourse import bass2jax
    from jax.experimental.shard_map import shard_map
    from jax.sharding import Mesh, NamedSharding, PartitionSpec

    nc = _get_nc(mm_dt, **build_kw)
    in_maps = make_in_maps(x, w_attn, b_attn, w_proj, b_proj, mm_dt)
    bass2jax.install_neuronx_cc_hook()

    partition_name = (
        nc.partition_id_tensor.name if nc.partition_id_tensor else None
    )
    in_names, out_names, out_avals, zero_outs = [], [], [], []
    for alloc in nc.m.functions[0].allocations:
        if not isinstance(alloc, mb.MemoryLocationSet):
            continue
        name = alloc.memorylocations[0].name
        if alloc.kind == "ExternalInput":
            if name != partition_name:
                in_names.append(name)
        elif alloc.kind == "ExternalOutput":
            out_names.append(name)
            shape = tuple(alloc.tensor_shape)
            dtype = mb.dt.np(alloc.dtype)
            out_avals.append(jax.core.ShapedArray(shape, dtype))
            zero_outs.append(np.zeros(shape, dtype))
    n_params = len(in_names)
    all_names = in_names + out_names
    if partition_name is not None:
        all_names = all_names + [partition_name]

    def _body(*args):
        operands = list(args)
        if partition_name is not None:
            operands.append(bass2jax.partition_id_tensor())
        outs = bass2jax._bass_exec_p.bind(
            *operands,
            out_avals=tuple(out_avals),
            in_names=tuple(all_names),
            out_names=tuple(out_names),
            lowering_input_output_aliases=(),
            sim_require_finite=True,
            sim_require_nnan=True,
            nc=nc,
        )
        return tuple(outs)

    devices = jax.devices()[:n_cores]
    mesh = Mesh(np.asarray(devices), ("core",))
    spec = PartitionSpec("core")
    f = jax.jit(
        shard_map(
            _body, mesh=mesh,
            in_specs=(spec,) * (n_params + len(out_names)),
            out_specs=(spec,) * len(out_names),
            check_rep=False,
        ),
        keep_unused=True,
    )
    sharding = NamedSharding(mesh, spec)
    args = [
        jax.device_put(
            np.concatenate([np.asarray(m[nm]) for m in in_maps], axis=0),
            sharding,
        )
        for nm in in_names
    ] + [
        jax.device_put(
            np.zeros((n_cores * z.shape[0], *z.shape[1:]), z.dtype), sharding
        )
        for z in zero_outs
    ]
    return f, args, out_names


def bench(x, w_attn, b_attn, w_proj, b_proj, iters=30, mm_dt=MM_DT,
          **build_kw):
    import time

    import jax

    f, args, out_names = make_bench(x, w_attn, b_attn, w_proj, b_proj, mm_dt,
                                    **build_kw)
    out = f(*args)  # compile + warm
    jax.block_until_ready(out)
    times = []
    for _ in range(iters):
        t0 = time.perf_counter()
        out = f(*args)
        jax.block_until_ready(out)
        times.append(time.perf_counter() - t0)
    times.sort()
    y_all = np.asarray(out[out_names.index("y")]).reshape(8, T, C)
    y = _combine([y_all[c] for c in range(8)], b_attn, w_proj, b_proj)
    return y, times

